# revision 28
# baseline (speedup 1.0000x reference)
"""Bass/Trainium2 kernel for nn_AttBlockMinimal (block-diagonal masked attention).

reference semantics:
    q = h @ W_h; k = f @ W_key; v = f @ W_value
    scores = q @ k.T  masked to -1e9 where x_batch[i] != f_batch[j] (block diagonal,
    both batch vectors sorted), softmax over keys, context = attn @ v.
    Returns (context [8192,256], attn [8192,8192]).

Masked softmax entries underflow to exactly 0.0, so attn is block-diagonal with
16 dense blocks (one per graph). Strategy: graph-parallel over 8 cores, 2 graphs
per core. The host slices/pads/transposes each graph's rows, every core runs the
same SPMD program on its two dense blocks, and the host scatters the dense
results into the (mostly zero) full outputs.

Device program per block (all matmuls contract over the partition dim):
    kT = W_key.T @ fT        [128d, NFB]
    qT = W_h.T  @ hT         [128d, NXB]
    v  = fT.T   @ W_value    [NFB, 257]    bf16, col 256 = 1 (gives ctx row sums)
    sT = kT.T-slices @ qT    [NFB, NXB]    (scores, transposed)
    e' = exp(sT - C_b)       bf16          (C_b = exact block max score, host side)
    rowsum = ones.T @ e'     [1, NXB], partition-broadcast to [128, NXB] (GpSimd)
    attnT  = e' / bcast(rowsum)  -> DRAM (host transposes per block)
    ctx_ps = e'.T @ v_aug;  ctx = ctx_ps[:, :256] * recip(ctx_ps[:, 256])

rmode: "fp32" - score path in fp32 (4 cyc/row), most accurate
       "sT"   - score matmul in f32r (~tf32), projections fp32
       "all"  - projections + score matmul in f32r
"""

import os
import numpy as np
import ml_dtypes

NX, NF = 8192, 8192
F_SIZE, OUT_SIZE, D_ATT = 512, 256, 128
N_GRAPHS = 16
NCORES = 8
NB = 2  # graph blocks per core
RMODE = os.environ.get("KERNEL_RMODE", "i16r")
N_WARM = 28  # HAM warm-up matmuls

_compiled_cache = {}


def _round_up(v, m):
    return ((v + m - 1) // m) * m


def _nchunks(n, c=512):
    out = []
    n0 = 0
    while n0 < n:
        nn = min(c, n - n0)
        out.append((n0, nn))
        n0 += nn
    return out


def _nchunks_ps(n):
    """(src_off, width, psum_off) chunks: widths >=256 when possible, each
    chunk placed at a 512-aligned psum offset so no matmul crosses a bank."""
    if n <= 512:
        return [(0, n, 0)]
    out = []
    n0 = 0
    ps = 0
    while n0 < n:
        rem = n - n0
        if rem > 512:
            nn = max(256, min(512, rem - 256))
        else:
            nn = rem
        out.append((n0, nn, ps))
        n0 += nn
        ps += 512
    return out


def _build(NXB, NFB, NX_IO, NF_IO, rmode):
    import concourse.bacc as bacc
    import concourse.mybir as mybir
    from concourse import tile

    f32 = mybir.dt.float32
    f32r = mybir.dt.float32r
    bf16 = mybir.dt.bfloat16
    i16 = mybir.dt.int16
    Exp = mybir.ActivationFunctionType.Exp
    mult = mybir.AluOpType.mult
    divide = mybir.AluOpType.divide

    f16 = mybir.dt.float16
    if rmode == "f16":
        in_dt = proj_dt = score_dt = val_dt = f16
    elif rmode == "i16r":
        in_dt = i16
        proj_dt = score_dt = val_dt = f32r
    else:
        in_dt = proj_dt = f32r if rmode == "all" else f32
        score_dt = f32r if rmode in ("sT", "all") else f32
        val_dt = bf16

    NJT = NFB // 128  # key tiles per block
    NIC = NXB // 128  # query chunks per block
    VW = OUT_SIZE + 1  # value width + ones column
    nchX = _nchunks(NXB)
    nchXps = _nchunks_ps(NXB)
    nchFps = _nchunks_ps(NFB)
    PSW = max(nchXps[-1][2] + 512, nchFps[-1][2] + 512)

    nc = bacc.Bacc("TRN2", target_bir_lowering=False, debug=False,
                   num_devices=NCORES)
    hqT_d = nc.dram_tensor("hqT", [OUT_SIZE, NB * NXB], in_dt, kind="ExternalInput")
    fkT_d = nc.dram_tensor("fkT", [F_SIZE, NB * NFB], in_dt, kind="ExternalInput")
    wh_d = nc.dram_tensor("w_h", [OUT_SIZE, D_ATT], proj_dt, kind="ExternalInput")
    wk_d = nc.dram_tensor("w_key", [F_SIZE, D_ATT], proj_dt, kind="ExternalInput")
    wv_d = nc.dram_tensor("w_vh", [F_SIZE, OUT_SIZE], val_dt, kind="ExternalInput")
    cb_d = nc.dram_tensor("cbias", [128, NB], f32, kind="ExternalInput")
    id_d = nc.dram_tensor("ident", [128, 128], f32, kind="ExternalInput")
    attnT_d = nc.dram_tensor("attnT", [NB * NFB, NXB], bf16, kind="ExternalOutput")
    ctx_d = nc.dram_tensor("ctx", [NB * NXB, OUT_SIZE], bf16, kind="ExternalOutput")

    def dma_3d(dst_tile, src, nt, width, b):
        # one DMA: dram [(nt*128), NB*width] block-column b -> sbuf [128, nt*width]
        nc.sync.dma_start(
            out=dst_tile[:].rearrange("p (t n) -> p t n", t=nt),
            in_=src.rearrange("(t p) n -> p t n", p=128)[:, :, b * width:(b + 1) * width])

    with tile.TileContext(nc) as tc:
        with (
            tc.tile_pool(name="const", bufs=1) as constp,
            tc.tile_pool(name="io", bufs=2) as iop,
            tc.tile_pool(name="work", bufs=2) as workp,
            tc.tile_pool(name="out", bufs=2) as outp,
            tc.tile_pool(name="atp", bufs=5) as atp,
            tc.tile_pool(name="psS", bufs=2, space="PSUM") as psS,
            tc.tile_pool(name="psV", bufs=1, space="PSUM") as psV,
            tc.tile_pool(name="psCx", bufs=3, space="PSUM") as psCx,
        ):
            # HAM warm-up burst from a memset tile (no DMA dependency)
            warm_sb = constp.tile([128, 128], bf16, tag="warm")
            nc.vector.memset(warm_sb[:], 1.0)
            w_ps = psS.tile([128, PSW], f32, tag="big")
            for w in range(N_WARM):
                nc.tensor.matmul(w_ps[:, 0:128], lhsT=warm_sb[:], rhs=warm_sb[:],
                                 start=True, stop=True)

            wk_sb = constp.tile([128, 4 * 128], proj_dt, tag="wk")
            wh_sb = constp.tile([128, 2 * 128], proj_dt, tag="wh")
            wv_sb = constp.tile([128, 4 * 256], val_dt, tag="wv")
            cb_sb = constp.tile([128, NB], f32, tag="cb")
            id_sb = constp.tile([128, 128], f32, tag="ident")
            ones_b = constp.tile([1, 128], bf16, tag="onesb")
            nc.vector.memset(ones_b[:], 1.0)

            for b in range(NB):
                # int16 inputs: half the DMA bytes of f32r at better-than-bf16
                # precision; dequant scale is folded into the weights host-side
                fkI_sb = iop.tile([128, 4 * NF_IO], in_dt, tag="fkI")
                nc.sync.dma_start(
                    out=fkI_sb[:].rearrange("p (t n) -> p t n", t=4),
                    in_=fkT_d.rearrange("(t p) n -> p t n", p=128)
                        [:, :, b * NFB:b * NFB + NF_IO])
                if b == 0:
                    # small weights ride behind the critical first key input
                    nc.sync.dma_start(
                        out=wk_sb[:].rearrange("p (t n) -> p t n", t=4),
                        in_=wk_d.rearrange("(t p) n -> p t n", p=128))
                hqI_sb = iop.tile([128, 2 * NX_IO], in_dt, tag="hqI")
                nc.sync.dma_start(
                    out=hqI_sb[:].rearrange("p (t n) -> p t n", t=2),
                    in_=hqT_d.rearrange("(t p) n -> p t n", p=128)
                        [:, :, b * NXB:b * NXB + NX_IO])
                if b == 0:
                    nc.sync.dma_start(
                        out=wh_sb[:].rearrange("p (t n) -> p t n", t=2),
                        in_=wh_d.rearrange("(t p) n -> p t n", p=128))
                    nc.sync.dma_start(out=cb_sb[:, :], in_=cb_d[:, :])
                    nc.sync.dma_start(
                        out=wv_sb[:].rearrange("p (t n) -> p t n", t=4),
                        in_=wv_d.rearrange("(t p) n -> p t n", p=128))
                    nc.sync.dma_start(out=id_sb[:], in_=id_d[:, :])
                # dequantize to f32r (the int16->float convert rounds; the
                # quant scale is folded into the weights host-side)
                fkT_sb = iop.tile([128, 4 * NFB], proj_dt, tag="fkT")
                for t in range(4):
                    nc.vector.tensor_copy(
                        fkT_sb[:, t * NFB:t * NFB + NF_IO],
                        fkI_sb[:, t * NF_IO:(t + 1) * NF_IO])
                hqT_sb = iop.tile([128, 2 * NXB], proj_dt, tag="hqT")
                nc.vector.tensor_copy(
                    hqT_sb[:].rearrange("p (t n) -> p t n", t=2)[:, :, 0:NX_IO],
                    hqI_sb[:].rearrange("p (t n) -> p t n", t=2))
                if NF_IO < NFB:
                    for t in range(4):
                        nc.vector.memset(
                            fkT_sb[:, t * NFB + NF_IO:(t + 1) * NFB].bitcast(f32), 0.0)
                if NX_IO < NXB:
                    for t in range(2):
                        nc.vector.memset(
                            hqT_sb[:, t * NXB + NX_IO:(t + 1) * NXB].bitcast(f32), 0.0)

                v_sb = workp.tile([128, NJT * VW], bf16, tag="v")
                eT_sb = workp.tile([128, NJT * NXB], bf16, tag="eT")

                def v_proj(jt):
                    v_ps = psV.tile([128, VW], f32, tag="vps")
                    for t in range(4):
                        nc.tensor.matmul(
                            v_ps[:, 0:256],
                            lhsT=fkT_sb[:, t * NFB + jt * 128:t * NFB + jt * 128 + 128],
                            rhs=wv_sb[:, t * 256:(t + 1) * 256],
                            start=(t == 0), stop=(t == 3))
                    nc.vector.tensor_copy(v_sb[:, jt * VW:jt * VW + 256],
                                          v_ps[:, 0:256])
                    nc.vector.memset(v_sb[:, jt * VW + 256:jt * VW + VW], 1.0)

                # key projection, transposed: kT[d, j]
                kT_ps = psS.tile([128, PSW], f32, tag="big")
                for t in range(4):
                    for n0, nn, ps in nchFps:
                        nc.tensor.matmul(
                            kT_ps[:, ps:ps + nn],
                            lhsT=wk_sb[:, t * 128:(t + 1) * 128],
                            rhs=fkT_sb[:, t * NFB + n0:t * NFB + n0 + nn],
                            start=(t == 0), stop=(t == 3))
                kT_sb = workp.tile([128, NFB], score_dt, tag="kT")
                for n0, nn, ps in nchFps:
                    nc.scalar.copy(kT_sb[:, n0:n0 + nn], kT_ps[:, ps:ps + nn])
                v_proj(0)  # fills PE while the kT copy drains

                # query projection, transposed: qT[d, i]
                qT_ps = psS.tile([128, PSW], f32, tag="big")
                for t in range(2):
                    for n0, nn, ps in nchXps:
                        nc.tensor.matmul(
                            qT_ps[:, ps:ps + nn],
                            lhsT=wh_sb[:, t * 128:(t + 1) * 128],
                            rhs=hqT_sb[:, t * NXB + n0:t * NXB + n0 + nn],
                            start=(t == 0), stop=(t == 1))
                qT_sb = workp.tile([128, NXB], score_dt, tag="qT")
                for n0, nn, ps in nchXps:
                    nc.scalar.copy(qT_sb[:, n0:n0 + nn], qT_ps[:, ps:ps + nn])
                v_proj(1)

                # scores + exp per key tile, with the context accumulation
                # (previous tile) and value projection interleaved on PE
                NIC_IN = min(3, NIC)  # ctx columns accumulated inside the loop
                c_ps = [psCx.tile([128, VW], f32, tag="ctx", name=f"cps{b}_{i}")
                        for i in range(NIC_IN)]
                for jt in range(NJT):
                    sT_ps = psS.tile([128, PSW], f32, tag="big")
                    for n0, nn, ps in nchXps:
                        nc.tensor.matmul(
                            sT_ps[:, ps:ps + nn],
                            lhsT=kT_sb[:, jt * 128:(jt + 1) * 128],
                            rhs=qT_sb[:, n0:n0 + nn],
                            start=True, stop=True)
                    for n0, nn, ps in nchXps:
                        nc.scalar.activation(
                            eT_sb[:, jt * NXB + n0:jt * NXB + n0 + nn],
                            sT_ps[:, ps:ps + nn],
                            Exp, bias=cb_sb[:, b:b + 1], scale=1.0)
                    if jt + 2 < NJT:
                        v_proj(jt + 2)
                    for ic in range(NIC_IN):
                        nc.tensor.matmul(
                            c_ps[ic][:, 0:VW],
                            lhsT=eT_sb[:, jt * NXB + ic * 128:jt * NXB + ic * 128 + 128],
                            rhs=v_sb[:, jt * VW:(jt + 1) * VW],
                            start=(jt == 0), stop=(jt == NJT - 1))
                for ic in range(NIC_IN, NIC):
                    cp = psCx.tile([128, VW], f32, tag="ctx", name=f"cpsl{b}_{ic}")
                    c_ps.append(cp)
                    for jt in range(NJT):
                        nc.tensor.matmul(
                            cp[:, 0:VW],
                            lhsT=eT_sb[:, jt * NXB + ic * 128:jt * NXB + ic * 128 + 128],
                            rhs=v_sb[:, jt * VW:(jt + 1) * VW],
                            start=(jt == 0), stop=(jt == NJT - 1))

                # context epilogue: per-row reciprocal of the ones-column sum
                cx_all = outp.tile([128, NIC * 256], bf16, tag="cx")
                rc_all = workp.tile([128, NIC], f32, tag="rc")
                for ic in range(NIC):
                    nc.vector.reciprocal(rc_all[:, ic:ic + 1], c_ps[ic][:, 256:VW])
                    nc.vector.tensor_scalar(cx_all[:, ic * 256:(ic + 1) * 256],
                                            c_ps[ic][:, 0:256], rc_all[:, ic:ic + 1],
                                            None, mult)
                nc.scalar.dma_start(
                    out=ctx_d[b * NXB:(b + 1) * NXB, :].rearrange(
                        "(t p) n -> p t n", p=128),
                    in_=cx_all[:].rearrange("p (t n) -> p t n", t=NIC))

                # transpose per-query recips into a [1, NXB] row via PE,
                # broadcast across partitions, multiply, ship per key tile
                n1 = min(512, NXB)
                rcT_ps1 = psV.tile([1, n1], f32, tag="vps")
                rcT_ps2 = (psCx.tile([1, NXB - n1], f32, tag="ctx",
                                     name=f"rcT2_{b}")
                           if NXB > n1 else None)
                for ic in range(NIC):
                    if ic * 128 < n1:
                        dst = rcT_ps1[0:1, ic * 128:(ic + 1) * 128]
                    else:
                        dst = rcT_ps2[0:1, ic * 128 - n1:(ic + 1) * 128 - n1]
                    nc.tensor.matmul(dst, lhsT=rc_all[:, ic:ic + 1], rhs=id_sb[:],
                                     is_transpose=True, start=True, stop=True)
                rcT_sb = workp.tile([1, NXB], bf16, tag="rcT")
                nc.vector.tensor_copy(rcT_sb[0:1, 0:n1], rcT_ps1[:])
                if rcT_ps2 is not None:
                    nc.vector.tensor_copy(rcT_sb[0:1, n1:NXB], rcT_ps2[:])
                bc_ps = psS.tile([128, PSW], f32, tag="big")
                for n0, nn, ps in nchXps:
                    nc.tensor.matmul(bc_ps[:, ps:ps + nn], lhsT=ones_b[:],
                                     rhs=rcT_sb[0:1, n0:n0 + nn],
                                     start=True, stop=True)
                bc_sb = workp.tile([128, NXB], bf16, tag="bc")
                for n0, nn, ps in nchXps:
                    nc.scalar.copy(bc_sb[:, n0:n0 + nn], bc_ps[:, ps:ps + nn])
                for jt in range(NJT):
                    at_sb = atp.tile([128, NXB], bf16, tag="at")
                    nc.vector.tensor_tensor(
                        at_sb[:, 0:NX_IO], eT_sb[:, jt * NXB:jt * NXB + NX_IO],
                        bc_sb[:, 0:NX_IO], mult)
                    nc.scalar.dma_start(
                        out=attnT_d[b * NFB + jt * 128:b * NFB + (jt + 1) * 128,
                                    0:NX_IO],
                        in_=at_sb[:, 0:NX_IO])

    nc.compile()
    return nc


def _get_compiled(NXB, NFB, NX_IO, NF_IO):
    key = (NXB, NFB, NX_IO, NF_IO, RMODE)
    if key not in _compiled_cache:
        _compiled_cache[key] = _build(NXB, NFB, NX_IO, NF_IO, RMODE)
    return _compiled_cache[key]


def kernel(f, x, h, edge_index, edge_type, f_batch, x_batch, x_init,
           W_h, W_key, W_value):
    from concourse.bass_utils import run_bass_kernel_spmd

    f = np.ascontiguousarray(np.asarray(f, dtype=np.float32))
    h = np.ascontiguousarray(np.asarray(h, dtype=np.float32))
    W_h = np.ascontiguousarray(np.asarray(W_h, dtype=np.float32))
    W_key = np.ascontiguousarray(np.asarray(W_key, dtype=np.float32))
    W_value = np.ascontiguousarray(np.asarray(W_value, dtype=np.float32))
    xb = np.asarray(x_batch).astype(np.int64)
    fb = np.asarray(f_batch).astype(np.int64)

    gids = np.arange(N_GRAPHS)
    xs = np.searchsorted(xb, gids, side="left")
    xe = np.searchsorted(xb, gids, side="right")
    fs = np.searchsorted(fb, gids, side="left")
    fe = np.searchsorted(fb, gids, side="right")
    nx_g = xe - xs
    nf_g = fe - fs

    NXB = _round_up(max(int(nx_g.max()), 1), 128)
    NFB = _round_up(max(int(nf_g.max()), 1), 128)
    NX_IO = _round_up(max(int(nx_g.max()), 1), 16)
    NF_IO = _round_up(max(int(nf_g.max()), 1), 16)

    # exact per-block score max for the exp shift (cheap on host via BLAS)
    q_host = h @ W_h
    k_host = f @ W_key
    cmax = np.zeros(N_GRAPHS, dtype=np.float32)
    for g in range(N_GRAPHS):
        if nx_g[g] > 0 and nf_g[g] > 0:
            cmax[g] = (q_host[xs[g]:xe[g]] @ k_host[fs[g]:fe[g]].T).max()

    nc = _get_compiled(NXB, NFB, NX_IO, NF_IO)

    scale_f = np.float32(max(np.abs(f).max(), 1e-30) / 32767.0)
    scale_h = np.float32(max(np.abs(h).max(), 1e-30) / 32767.0)
    f_q = np.round(f / scale_f).astype(np.int16)
    h_q = np.round(h / scale_h).astype(np.int16)
    wk_q = W_key * scale_f
    wh_q = W_h * scale_h
    wv_q = W_value * scale_f
    # the exp shift must track the quantized scores the device computes
    q_q = (h_q.astype(np.float32) * scale_h) @ W_h
    k_q = (f_q.astype(np.float32) * scale_f) @ W_key
    for g in range(N_GRAPHS):
        if nx_g[g] > 0 and nf_g[g] > 0:
            cmax[g] = (q_q[xs[g]:xe[g]] @ k_q[fs[g]:fe[g]].T).max()

    in_maps = []
    ident = np.eye(128, dtype=np.float32)
    for c in range(NCORES):
        hqT = np.zeros((OUT_SIZE, NB * NXB), np.int16)
        fkT = np.zeros((F_SIZE, NB * NFB), np.int16)
        cb = np.zeros((128, NB), np.float32)
        for b in range(NB):
            g = NB * c + b
            hqT[:, b * NXB:b * NXB + nx_g[g]] = h_q[xs[g]:xe[g]].T
            fkT[:, b * NFB:b * NFB + nf_g[g]] = f_q[fs[g]:fe[g]].T
            cb[:, b] = -cmax[g]
        in_maps.append({
            "hqT": hqT,
            "fkT": fkT,
            "w_h": wh_q,
            "w_key": wk_q,
            "w_vh": wv_q,
            "cbias": cb,
            "ident": ident,
        })

    res = run_bass_kernel_spmd(nc, in_maps, list(range(NCORES)))
    kernel._last_results = res

    attn = np.zeros((NX, NF), np.float32)
    ctx = np.zeros((NX, OUT_SIZE), np.float32)
    value_host = None
    for c in range(NCORES):
        at = np.asarray(res.results[c]["attnT"])
        cx = np.asarray(res.results[c]["ctx"])
        for b in range(NB):
            g = NB * c + b
            if nx_g[g] == 0:
                continue
            if nf_g[g] > 0:
                attn[xs[g]:xe[g], fs[g]:fe[g]] = \
                    at[b * NFB:b * NFB + nf_g[g], :nx_g[g]].T
                ctx[xs[g]:xe[g], :] = cx[b * NXB:b * NXB + nx_g[g], :]
            else:
                # graph with queries but no keys: reference softmax over an
                # all -1e9 row is uniform over ALL keys
                attn[xs[g]:xe[g], :] = np.float32(1.0) / np.float32(NF)
                if value_host is None:
                    value_host = f @ W_value
                ctx[xs[g]:xe[g], :] = value_host.mean(axis=0, dtype=np.float32)
    return ctx, attn


# revision 29
# speedup vs baseline: 1.1350x; 1.1350x over previous
"""Bass/Trainium2 kernel for nn_AttBlockMinimal (block-diagonal masked attention).

reference semantics:
    q = h @ W_h; k = f @ W_key; v = f @ W_value
    scores = q @ k.T  masked to -1e9 where x_batch[i] != f_batch[j] (block diagonal,
    both batch vectors sorted), softmax over keys, context = attn @ v.
    Returns (context [8192,256], attn [8192,8192]).

Masked softmax entries underflow to exactly 0.0, so attn is block-diagonal with
16 dense blocks (one per graph). Strategy: graph-parallel over 8 cores, 2 graphs
per core. The host slices/pads/transposes each graph's rows, every core runs the
same SPMD program on its two dense blocks, and the host scatters the dense
results into the (mostly zero) full outputs.

Device program per block (all matmuls contract over the partition dim):
    kT = W_key.T @ fT        [128d, NFB]
    qT = W_h.T  @ hT         [128d, NXB]
    v  = fT.T   @ W_value    [NFB, 257]    bf16, col 256 = 1 (gives ctx row sums)
    sT = kT.T-slices @ qT    [NFB, NXB]    (scores, transposed)
    e' = exp(sT - C_b)       bf16          (C_b = exact block max score, host side)
    rowsum = ones.T @ e'     [1, NXB], partition-broadcast to [128, NXB] (GpSimd)
    attnT  = e' / bcast(rowsum)  -> DRAM (host transposes per block)
    ctx_ps = e'.T @ v_aug;  ctx = ctx_ps[:, :256] * recip(ctx_ps[:, 256])

rmode: "fp32" - score path in fp32 (4 cyc/row), most accurate
       "sT"   - score matmul in f32r (~tf32), projections fp32
       "all"  - projections + score matmul in f32r
"""

import os
import numpy as np
import ml_dtypes

NX, NF = 8192, 8192
F_SIZE, OUT_SIZE, D_ATT = 512, 256, 128
N_GRAPHS = 16
NCORES = 8
NB = 2  # graph blocks per core
RMODE = os.environ.get("KERNEL_RMODE", "i16r")
N_WARM = 28  # HAM warm-up matmuls

_compiled_cache = {}


def _round_up(v, m):
    return ((v + m - 1) // m) * m


def _nchunks(n, c=512):
    out = []
    n0 = 0
    while n0 < n:
        nn = min(c, n - n0)
        out.append((n0, nn))
        n0 += nn
    return out


def _nchunks_ps(n):
    """(src_off, width, psum_off) chunks: widths >=256 when possible, each
    chunk placed at a 512-aligned psum offset so no matmul crosses a bank."""
    if n <= 512:
        return [(0, n, 0)]
    out = []
    n0 = 0
    ps = 0
    while n0 < n:
        rem = n - n0
        if rem > 512:
            nn = max(256, min(512, rem - 256))
        else:
            nn = rem
        out.append((n0, nn, ps))
        n0 += nn
        ps += 512
    return out


def _build(NXB, NFB, NX_IO, NF_IO, rmode):
    import concourse.bacc as bacc
    import concourse.mybir as mybir
    from concourse import tile

    f32 = mybir.dt.float32
    f32r = mybir.dt.float32r
    bf16 = mybir.dt.bfloat16
    i16 = mybir.dt.int16
    Exp = mybir.ActivationFunctionType.Exp
    mult = mybir.AluOpType.mult
    divide = mybir.AluOpType.divide

    f16 = mybir.dt.float16
    if rmode == "f16":
        in_dt = proj_dt = score_dt = val_dt = f16
    elif rmode == "i16r":
        in_dt = i16
        proj_dt = score_dt = val_dt = f32r
    else:
        in_dt = proj_dt = f32r if rmode == "all" else f32
        score_dt = f32r if rmode in ("sT", "all") else f32
        val_dt = bf16

    NJT = NFB // 128  # key tiles per block
    NIC = NXB // 128  # query chunks per block
    VW = OUT_SIZE + 1  # value width + ones column
    nchX = _nchunks(NXB)
    nchXps = _nchunks_ps(NXB)
    nchFps = _nchunks_ps(NFB)
    PSW = max(nchXps[-1][2] + 512, nchFps[-1][2] + 512)

    nc = bacc.Bacc("TRN2", target_bir_lowering=False, debug=False,
                   num_devices=NCORES)
    hqT_d = nc.dram_tensor("hqT", [OUT_SIZE, NB * NXB], in_dt, kind="ExternalInput")
    fkT_d = nc.dram_tensor("fkT", [F_SIZE, NB * NFB], in_dt, kind="ExternalInput")
    wh_d = nc.dram_tensor("w_h", [OUT_SIZE, D_ATT], proj_dt, kind="ExternalInput")
    wk_d = nc.dram_tensor("w_key", [F_SIZE, D_ATT], proj_dt, kind="ExternalInput")
    wv_d = nc.dram_tensor("w_vh", [F_SIZE, OUT_SIZE], val_dt, kind="ExternalInput")
    cb_d = nc.dram_tensor("cbias", [128, NB], f32, kind="ExternalInput")
    id_d = nc.dram_tensor("ident", [128, 128], f32, kind="ExternalInput")
    attnT_d = nc.dram_tensor("attnT", [NB * NFB, NXB], bf16, kind="ExternalOutput")
    ctx_d = nc.dram_tensor("ctx", [NB * NXB, OUT_SIZE], bf16, kind="ExternalOutput")

    def dma_3d(dst_tile, src, nt, width, b):
        # one DMA: dram [(nt*128), NB*width] block-column b -> sbuf [128, nt*width]
        nc.sync.dma_start(
            out=dst_tile[:].rearrange("p (t n) -> p t n", t=nt),
            in_=src.rearrange("(t p) n -> p t n", p=128)[:, :, b * width:(b + 1) * width])

    with tile.TileContext(nc) as tc:
        with (
            tc.tile_pool(name="const", bufs=1) as constp,
            tc.tile_pool(name="io", bufs=2) as iop,
            tc.tile_pool(name="work", bufs=2) as workp,
            tc.tile_pool(name="out", bufs=2) as outp,
            tc.tile_pool(name="atp", bufs=5) as atp,
            tc.tile_pool(name="psS", bufs=2, space="PSUM") as psS,
            tc.tile_pool(name="psV", bufs=1, space="PSUM") as psV,
            tc.tile_pool(name="psCx", bufs=3, space="PSUM") as psCx,
        ):
            # HAM warm-up burst from a memset tile (no DMA dependency)
            warm_sb = constp.tile([128, 128], bf16, tag="warm")
            nc.vector.memset(warm_sb[:], 1.0)
            w_ps = psS.tile([128, PSW], f32, tag="big")
            for w in range(N_WARM):
                nc.tensor.matmul(w_ps[:, 0:128], lhsT=warm_sb[:], rhs=warm_sb[:],
                                 start=True, stop=True)

            wk_sb = constp.tile([128, 4 * 128], proj_dt, tag="wk")
            wh_sb = constp.tile([128, 2 * 128], proj_dt, tag="wh")
            wv_sb = constp.tile([128, 4 * 256], val_dt, tag="wv")
            cb_sb = constp.tile([128, NB], f32, tag="cb")
            id_sb = constp.tile([128, 128], f32, tag="ident")
            ones_b = constp.tile([1, 128], bf16, tag="onesb")
            nc.vector.memset(ones_b[:], 1.0)

            for b in range(NB):
                # int16 inputs: half the DMA bytes of f32r at better-than-bf16
                # precision; dequant scale is folded into the weights host-side
                fkI_sb = iop.tile([128, 4 * NF_IO], in_dt, tag="fkI")
                nc.sync.dma_start(
                    out=fkI_sb[:].rearrange("p (t n) -> p t n", t=4),
                    in_=fkT_d.rearrange("(t p) n -> p t n", p=128)
                        [:, :, b * NFB:b * NFB + NF_IO])
                if b == 0:
                    # small weights ride behind the critical first key input
                    nc.sync.dma_start(
                        out=wk_sb[:].rearrange("p (t n) -> p t n", t=4),
                        in_=wk_d.rearrange("(t p) n -> p t n", p=128))
                hqI_sb = iop.tile([128, 2 * NX_IO], in_dt, tag="hqI")
                nc.sync.dma_start(
                    out=hqI_sb[:].rearrange("p (t n) -> p t n", t=2),
                    in_=hqT_d.rearrange("(t p) n -> p t n", p=128)
                        [:, :, b * NXB:b * NXB + NX_IO])
                if b == 0:
                    nc.sync.dma_start(
                        out=wh_sb[:].rearrange("p (t n) -> p t n", t=2),
                        in_=wh_d.rearrange("(t p) n -> p t n", p=128))
                    nc.sync.dma_start(out=cb_sb[:, :], in_=cb_d[:, :])
                    nc.sync.dma_start(
                        out=wv_sb[:].rearrange("p (t n) -> p t n", t=4),
                        in_=wv_d.rearrange("(t p) n -> p t n", p=128))
                    nc.sync.dma_start(out=id_sb[:], in_=id_d[:, :])
                # dequantize to f32r (the int16->float convert rounds; the
                # quant scale is folded into the weights host-side)
                fkT_sb = iop.tile([128, 4 * NFB], proj_dt, tag="fkT")
                for t in range(4):
                    nc.vector.tensor_copy(
                        fkT_sb[:, t * NFB:t * NFB + NF_IO],
                        fkI_sb[:, t * NF_IO:(t + 1) * NF_IO])
                hqT_sb = iop.tile([128, 2 * NXB], proj_dt, tag="hqT")
                nc.vector.tensor_copy(
                    hqT_sb[:].rearrange("p (t n) -> p t n", t=2)[:, :, 0:NX_IO],
                    hqI_sb[:].rearrange("p (t n) -> p t n", t=2))
                if NF_IO < NFB:
                    for t in range(4):
                        nc.vector.memset(
                            fkT_sb[:, t * NFB + NF_IO:(t + 1) * NFB].bitcast(f32), 0.0)
                if NX_IO < NXB:
                    for t in range(2):
                        nc.vector.memset(
                            hqT_sb[:, t * NXB + NX_IO:(t + 1) * NXB].bitcast(f32), 0.0)

                v_sb = workp.tile([128, NJT * VW], bf16, tag="v")
                eT_sb = workp.tile([128, NJT * NXB], bf16, tag="eT")

                def v_proj(jt):
                    v_ps = psV.tile([128, VW], f32, tag="vps")
                    for t in range(4):
                        nc.tensor.matmul(
                            v_ps[:, 0:256],
                            lhsT=fkT_sb[:, t * NFB + jt * 128:t * NFB + jt * 128 + 128],
                            rhs=wv_sb[:, t * 256:(t + 1) * 256],
                            start=(t == 0), stop=(t == 3))
                    nc.vector.tensor_copy(v_sb[:, jt * VW:jt * VW + 256],
                                          v_ps[:, 0:256])
                    nc.vector.memset(v_sb[:, jt * VW + 256:jt * VW + VW], 1.0)

                # key projection, transposed: kT[d, j]
                kT_ps = psS.tile([128, PSW], f32, tag="big")
                for t in range(4):
                    for n0, nn, ps in nchFps:
                        nc.tensor.matmul(
                            kT_ps[:, ps:ps + nn],
                            lhsT=wk_sb[:, t * 128:(t + 1) * 128],
                            rhs=fkT_sb[:, t * NFB + n0:t * NFB + n0 + nn],
                            start=(t == 0), stop=(t == 3))
                kT_sb = workp.tile([128, NFB], score_dt, tag="kT")
                for n0, nn, ps in nchFps:
                    nc.scalar.copy(kT_sb[:, n0:n0 + nn], kT_ps[:, ps:ps + nn])
                v_proj(0)  # fills PE while the kT copy drains

                # query projection, transposed: qT[d, i]
                qT_ps = psS.tile([128, PSW], f32, tag="big")
                for t in range(2):
                    for n0, nn, ps in nchXps:
                        nc.tensor.matmul(
                            qT_ps[:, ps:ps + nn],
                            lhsT=wh_sb[:, t * 128:(t + 1) * 128],
                            rhs=hqT_sb[:, t * NXB + n0:t * NXB + n0 + nn],
                            start=(t == 0), stop=(t == 1))
                qT_sb = workp.tile([128, NXB], score_dt, tag="qT")
                for n0, nn, ps in nchXps:
                    nc.scalar.copy(qT_sb[:, n0:n0 + nn], qT_ps[:, ps:ps + nn])
                v_proj(1)

                # scores + exp per key tile, with the context accumulation
                # (previous tile) and value projection interleaved on PE
                NIC_IN = min(3, NIC)  # ctx columns accumulated inside the loop
                c_ps = [psCx.tile([128, VW], f32, tag="ctx", name=f"cps{b}_{i}")
                        for i in range(NIC_IN)]
                for jt in range(NJT):
                    sT_ps = psS.tile([128, PSW], f32, tag="big")
                    for n0, nn, ps in nchXps:
                        nc.tensor.matmul(
                            sT_ps[:, ps:ps + nn],
                            lhsT=kT_sb[:, jt * 128:(jt + 1) * 128],
                            rhs=qT_sb[:, n0:n0 + nn],
                            start=True, stop=True)
                    for n0, nn, ps in nchXps:
                        nc.scalar.activation(
                            eT_sb[:, jt * NXB + n0:jt * NXB + n0 + nn],
                            sT_ps[:, ps:ps + nn],
                            Exp, bias=cb_sb[:, b:b + 1], scale=1.0)
                    if jt + 2 < NJT:
                        v_proj(jt + 2)
                    for ic in range(NIC_IN):
                        nc.tensor.matmul(
                            c_ps[ic][:, 0:VW],
                            lhsT=eT_sb[:, jt * NXB + ic * 128:jt * NXB + ic * 128 + 128],
                            rhs=v_sb[:, jt * VW:(jt + 1) * VW],
                            start=(jt == 0), stop=(jt == NJT - 1))
                for ic in range(NIC_IN, NIC):
                    cp = psCx.tile([128, VW], f32, tag="ctx", name=f"cpsl{b}_{ic}")
                    c_ps.append(cp)
                    for jt in range(NJT):
                        nc.tensor.matmul(
                            cp[:, 0:VW],
                            lhsT=eT_sb[:, jt * NXB + ic * 128:jt * NXB + ic * 128 + 128],
                            rhs=v_sb[:, jt * VW:(jt + 1) * VW],
                            start=(jt == 0), stop=(jt == NJT - 1))

                # context epilogue: per-row reciprocal of the ones-column sum
                cx_all = outp.tile([128, NIC * 256], bf16, tag="cx")
                rc_all = workp.tile([128, NIC], f32, tag="rc")
                for ic in range(NIC):
                    nc.vector.reciprocal(rc_all[:, ic:ic + 1], c_ps[ic][:, 256:VW])
                    nc.vector.tensor_scalar(cx_all[:, ic * 256:(ic + 1) * 256],
                                            c_ps[ic][:, 0:256], rc_all[:, ic:ic + 1],
                                            None, mult)
                nc.scalar.dma_start(
                    out=ctx_d[b * NXB:(b + 1) * NXB, :].rearrange(
                        "(t p) n -> p t n", p=128),
                    in_=cx_all[:].rearrange("p (t n) -> p t n", t=NIC))

                # transpose per-query recips into a [1, NXB] row via PE,
                # broadcast across partitions, multiply, ship per key tile
                n1 = min(512, NXB)
                rcT_ps1 = psV.tile([1, n1], f32, tag="vps")
                rcT_ps2 = (psCx.tile([1, NXB - n1], f32, tag="ctx",
                                     name=f"rcT2_{b}")
                           if NXB > n1 else None)
                for ic in range(NIC):
                    if ic * 128 < n1:
                        dst = rcT_ps1[0:1, ic * 128:(ic + 1) * 128]
                    else:
                        dst = rcT_ps2[0:1, ic * 128 - n1:(ic + 1) * 128 - n1]
                    nc.tensor.matmul(dst, lhsT=rc_all[:, ic:ic + 1], rhs=id_sb[:],
                                     is_transpose=True, start=True, stop=True)
                rcT_sb = workp.tile([1, NXB], bf16, tag="rcT")
                nc.vector.tensor_copy(rcT_sb[0:1, 0:n1], rcT_ps1[:])
                if rcT_ps2 is not None:
                    nc.vector.tensor_copy(rcT_sb[0:1, n1:NXB], rcT_ps2[:])
                bc_sb = workp.tile([128, NXB], bf16, tag="bc")
                nc.gpsimd.partition_broadcast(bc_sb[:], rcT_sb[:])
                for jt in range(NJT):
                    at_sb = atp.tile([128, NXB], bf16, tag="at")
                    nc.vector.tensor_tensor(
                        at_sb[:, 0:NX_IO], eT_sb[:, jt * NXB:jt * NXB + NX_IO],
                        bc_sb[:, 0:NX_IO], mult)
                    nc.scalar.dma_start(
                        out=attnT_d[b * NFB + jt * 128:b * NFB + (jt + 1) * 128,
                                    0:NX_IO],
                        in_=at_sb[:, 0:NX_IO])

    nc.compile()
    return nc


def _get_compiled(NXB, NFB, NX_IO, NF_IO):
    key = (NXB, NFB, NX_IO, NF_IO, RMODE)
    if key not in _compiled_cache:
        _compiled_cache[key] = _build(NXB, NFB, NX_IO, NF_IO, RMODE)
    return _compiled_cache[key]


def kernel(f, x, h, edge_index, edge_type, f_batch, x_batch, x_init,
           W_h, W_key, W_value):
    from concourse.bass_utils import run_bass_kernel_spmd

    f = np.ascontiguousarray(np.asarray(f, dtype=np.float32))
    h = np.ascontiguousarray(np.asarray(h, dtype=np.float32))
    W_h = np.ascontiguousarray(np.asarray(W_h, dtype=np.float32))
    W_key = np.ascontiguousarray(np.asarray(W_key, dtype=np.float32))
    W_value = np.ascontiguousarray(np.asarray(W_value, dtype=np.float32))
    xb = np.asarray(x_batch).astype(np.int64)
    fb = np.asarray(f_batch).astype(np.int64)

    gids = np.arange(N_GRAPHS)
    xs = np.searchsorted(xb, gids, side="left")
    xe = np.searchsorted(xb, gids, side="right")
    fs = np.searchsorted(fb, gids, side="left")
    fe = np.searchsorted(fb, gids, side="right")
    nx_g = xe - xs
    nf_g = fe - fs

    NXB = _round_up(max(int(nx_g.max()), 1), 128)
    NFB = _round_up(max(int(nf_g.max()), 1), 128)
    NX_IO = _round_up(max(int(nx_g.max()), 1), 16)
    NF_IO = _round_up(max(int(nf_g.max()), 1), 16)

    # exact per-block score max for the exp shift (cheap on host via BLAS)
    q_host = h @ W_h
    k_host = f @ W_key
    cmax = np.zeros(N_GRAPHS, dtype=np.float32)
    for g in range(N_GRAPHS):
        if nx_g[g] > 0 and nf_g[g] > 0:
            cmax[g] = (q_host[xs[g]:xe[g]] @ k_host[fs[g]:fe[g]].T).max()

    nc = _get_compiled(NXB, NFB, NX_IO, NF_IO)

    scale_f = np.float32(max(np.abs(f).max(), 1e-30) / 32767.0)
    scale_h = np.float32(max(np.abs(h).max(), 1e-30) / 32767.0)
    f_q = np.round(f / scale_f).astype(np.int16)
    h_q = np.round(h / scale_h).astype(np.int16)
    wk_q = W_key * scale_f
    wh_q = W_h * scale_h
    wv_q = W_value * scale_f
    # the exp shift must track the quantized scores the device computes
    q_q = (h_q.astype(np.float32) * scale_h) @ W_h
    k_q = (f_q.astype(np.float32) * scale_f) @ W_key
    for g in range(N_GRAPHS):
        if nx_g[g] > 0 and nf_g[g] > 0:
            cmax[g] = (q_q[xs[g]:xe[g]] @ k_q[fs[g]:fe[g]].T).max()

    in_maps = []
    ident = np.eye(128, dtype=np.float32)
    for c in range(NCORES):
        hqT = np.zeros((OUT_SIZE, NB * NXB), np.int16)
        fkT = np.zeros((F_SIZE, NB * NFB), np.int16)
        cb = np.zeros((128, NB), np.float32)
        for b in range(NB):
            g = NB * c + b
            hqT[:, b * NXB:b * NXB + nx_g[g]] = h_q[xs[g]:xe[g]].T
            fkT[:, b * NFB:b * NFB + nf_g[g]] = f_q[fs[g]:fe[g]].T
            cb[:, b] = -cmax[g]
        in_maps.append({
            "hqT": hqT,
            "fkT": fkT,
            "w_h": wh_q,
            "w_key": wk_q,
            "w_vh": wv_q,
            "cbias": cb,
            "ident": ident,
        })

    res = run_bass_kernel_spmd(nc, in_maps, list(range(NCORES)))
    kernel._last_results = res

    attn = np.zeros((NX, NF), np.float32)
    ctx = np.zeros((NX, OUT_SIZE), np.float32)
    value_host = None
    for c in range(NCORES):
        at = np.asarray(res.results[c]["attnT"])
        cx = np.asarray(res.results[c]["ctx"])
        for b in range(NB):
            g = NB * c + b
            if nx_g[g] == 0:
                continue
            if nf_g[g] > 0:
                attn[xs[g]:xe[g], fs[g]:fe[g]] = \
                    at[b * NFB:b * NFB + nf_g[g], :nx_g[g]].T
                ctx[xs[g]:xe[g], :] = cx[b * NXB:b * NXB + nx_g[g], :]
            else:
                # graph with queries but no keys: reference softmax over an
                # all -1e9 row is uniform over ALL keys
                attn[xs[g]:xe[g], :] = np.float32(1.0) / np.float32(NF)
                if value_host is None:
                    value_host = f @ W_value
                ctx[xs[g]:xe[g], :] = value_host.mean(axis=0, dtype=np.float32)
    return ctx, attn


# revision 30
# speedup vs baseline: 1.1547x; 1.0174x over previous
"""Bass/Trainium2 kernel for nn_AttBlockMinimal (block-diagonal masked attention).

reference semantics:
    q = h @ W_h; k = f @ W_key; v = f @ W_value
    scores = q @ k.T  masked to -1e9 where x_batch[i] != f_batch[j] (block diagonal,
    both batch vectors sorted), softmax over keys, context = attn @ v.
    Returns (context [8192,256], attn [8192,8192]).

Masked softmax entries underflow to exactly 0.0, so attn is block-diagonal with
16 dense blocks (one per graph). Strategy: graph-parallel over 8 cores, 2 graphs
per core. The host slices/pads/transposes each graph's rows, every core runs the
same SPMD program on its two dense blocks, and the host scatters the dense
results into the (mostly zero) full outputs.

Device program per block (all matmuls contract over the partition dim):
    kT = W_key.T @ fT        [128d, NFB]
    qT = W_h.T  @ hT         [128d, NXB]
    v  = fT.T   @ W_value    [NFB, 257]    bf16, col 256 = 1 (gives ctx row sums)
    sT = kT.T-slices @ qT    [NFB, NXB]    (scores, transposed)
    e' = exp(sT - C_b)       bf16          (C_b = exact block max score, host side)
    rowsum = ones.T @ e'     [1, NXB], partition-broadcast to [128, NXB] (GpSimd)
    attnT  = e' / bcast(rowsum)  -> DRAM (host transposes per block)
    ctx_ps = e'.T @ v_aug;  ctx = ctx_ps[:, :256] * recip(ctx_ps[:, 256])

rmode: "fp32" - score path in fp32 (4 cyc/row), most accurate
       "sT"   - score matmul in f32r (~tf32), projections fp32
       "all"  - projections + score matmul in f32r
"""

import os
import numpy as np
import ml_dtypes

NX, NF = 8192, 8192
F_SIZE, OUT_SIZE, D_ATT = 512, 256, 128
N_GRAPHS = 16
NCORES = 8
NB = 2  # graph blocks per core
RMODE = os.environ.get("KERNEL_RMODE", "i16r")
N_WARM = 28  # HAM warm-up matmuls

_compiled_cache = {}


def _round_up(v, m):
    return ((v + m - 1) // m) * m


def _nchunks(n, c=512):
    out = []
    n0 = 0
    while n0 < n:
        nn = min(c, n - n0)
        out.append((n0, nn))
        n0 += nn
    return out


def _nchunks_ps(n):
    """(src_off, width, psum_off) chunks: widths >=256 when possible, each
    chunk placed at a 512-aligned psum offset so no matmul crosses a bank."""
    if n <= 512:
        return [(0, n, 0)]
    out = []
    n0 = 0
    ps = 0
    while n0 < n:
        rem = n - n0
        if rem > 512:
            nn = max(256, min(512, rem - 256))
        else:
            nn = rem
        out.append((n0, nn, ps))
        n0 += nn
        ps += 512
    return out


def _build(NXB, NFB, NX_IO, NF_IO, rmode):
    import concourse.bacc as bacc
    import concourse.mybir as mybir
    from concourse import tile

    f32 = mybir.dt.float32
    f32r = mybir.dt.float32r
    bf16 = mybir.dt.bfloat16
    i16 = mybir.dt.int16
    Exp = mybir.ActivationFunctionType.Exp
    mult = mybir.AluOpType.mult
    divide = mybir.AluOpType.divide

    f16 = mybir.dt.float16
    if rmode == "f16":
        in_dt = proj_dt = score_dt = val_dt = f16
    elif rmode == "i16r":
        in_dt = i16
        proj_dt = score_dt = val_dt = f32r
    else:
        in_dt = proj_dt = f32r if rmode == "all" else f32
        score_dt = f32r if rmode in ("sT", "all") else f32
        val_dt = bf16

    NJT = NFB // 128  # key tiles per block
    NIC = NXB // 128  # query chunks per block
    VW = OUT_SIZE + 1  # value width + ones column
    nchX = _nchunks(NXB)
    nchXps = _nchunks_ps(NXB)
    nchFps = _nchunks_ps(NFB)
    PSW = max(nchXps[-1][2] + 512, nchFps[-1][2] + 512)

    nc = bacc.Bacc("TRN2", target_bir_lowering=False, debug=False,
                   num_devices=NCORES)
    hqT_d = nc.dram_tensor("hqT", [OUT_SIZE, NB * NXB], in_dt, kind="ExternalInput")
    fkT_d = nc.dram_tensor("fkT", [F_SIZE, NB * NFB], in_dt, kind="ExternalInput")
    wh_d = nc.dram_tensor("w_h", [OUT_SIZE, D_ATT], proj_dt, kind="ExternalInput")
    wk_d = nc.dram_tensor("w_key", [F_SIZE, D_ATT], proj_dt, kind="ExternalInput")
    wv_d = nc.dram_tensor("w_vh", [F_SIZE, OUT_SIZE], val_dt, kind="ExternalInput")
    cb_d = nc.dram_tensor("cbias", [128, NB], f32, kind="ExternalInput")
    id_d = nc.dram_tensor("ident", [128, 128], f32, kind="ExternalInput")
    attnT_d = nc.dram_tensor("attnT", [NB * NFB, NXB], bf16, kind="ExternalOutput")
    ctx_d = nc.dram_tensor("ctx", [NB * NXB, OUT_SIZE], bf16, kind="ExternalOutput")

    def dma_3d(dst_tile, src, nt, width, b):
        # one DMA: dram [(nt*128), NB*width] block-column b -> sbuf [128, nt*width]
        nc.sync.dma_start(
            out=dst_tile[:].rearrange("p (t n) -> p t n", t=nt),
            in_=src.rearrange("(t p) n -> p t n", p=128)[:, :, b * width:(b + 1) * width])

    with tile.TileContext(nc) as tc:
        with (
            tc.tile_pool(name="const", bufs=1) as constp,
            tc.tile_pool(name="io", bufs=2) as iop,
            tc.tile_pool(name="work", bufs=2) as workp,
            tc.tile_pool(name="out", bufs=2) as outp,
            tc.tile_pool(name="atp", bufs=5) as atp,
            tc.tile_pool(name="psS", bufs=2, space="PSUM") as psS,
            tc.tile_pool(name="psV", bufs=1, space="PSUM") as psV,
            tc.tile_pool(name="psCx", bufs=3, space="PSUM") as psCx,
        ):
            # HAM warm-up burst from a memset tile (no DMA dependency)
            warm_sb = constp.tile([128, 128], bf16, tag="warm")
            nc.vector.memset(warm_sb[:], 1.0)
            w_ps = psS.tile([128, PSW], f32, tag="big")
            for w in range(N_WARM):
                nc.tensor.matmul(w_ps[:, 0:128], lhsT=warm_sb[:], rhs=warm_sb[:],
                                 start=True, stop=True)

            wk_sb = constp.tile([128, 4 * 128], proj_dt, tag="wk")
            wh_sb = constp.tile([128, 2 * 128], proj_dt, tag="wh")
            wv_sb = constp.tile([128, 4 * 256], val_dt, tag="wv")
            cb_sb = constp.tile([128, NB], f32, tag="cb")
            id_sb = constp.tile([128, 128], f32, tag="ident")
            ones_b = constp.tile([1, 128], bf16, tag="onesb")
            nc.vector.memset(ones_b[:], 1.0)

            for b in range(NB):
                # int16 inputs: half the DMA bytes of f32r at better-than-bf16
                # precision; dequant scale is folded into the weights host-side
                fkI_sb = iop.tile([128, 4 * NF_IO], in_dt, tag="fkI")
                nc.sync.dma_start(
                    out=fkI_sb[:].rearrange("p (t n) -> p t n", t=4),
                    in_=fkT_d.rearrange("(t p) n -> p t n", p=128)
                        [:, :, b * NFB:b * NFB + NF_IO])
                if b == 0:
                    # small weights ride behind the critical first key input
                    nc.sync.dma_start(
                        out=wk_sb[:].rearrange("p (t n) -> p t n", t=4),
                        in_=wk_d.rearrange("(t p) n -> p t n", p=128))
                hqI_sb = iop.tile([128, 2 * NX_IO], in_dt, tag="hqI")
                nc.sync.dma_start(
                    out=hqI_sb[:].rearrange("p (t n) -> p t n", t=2),
                    in_=hqT_d.rearrange("(t p) n -> p t n", p=128)
                        [:, :, b * NXB:b * NXB + NX_IO])
                if b == 0:
                    nc.sync.dma_start(
                        out=wh_sb[:].rearrange("p (t n) -> p t n", t=2),
                        in_=wh_d.rearrange("(t p) n -> p t n", p=128))
                    nc.sync.dma_start(out=cb_sb[:, :], in_=cb_d[:, :])
                    nc.sync.dma_start(
                        out=wv_sb[:].rearrange("p (t n) -> p t n", t=4),
                        in_=wv_d.rearrange("(t p) n -> p t n", p=128))
                    nc.sync.dma_start(out=id_sb[:], in_=id_d[:, :])
                # dequantize to f32r (the int16->float convert rounds; the
                # quant scale is folded into the weights host-side)
                fkT_sb = iop.tile([128, 4 * NFB], proj_dt, tag="fkT")
                for t in range(4):
                    nc.vector.tensor_copy(
                        fkT_sb[:, t * NFB:t * NFB + NF_IO],
                        fkI_sb[:, t * NF_IO:(t + 1) * NF_IO])
                hqT_sb = iop.tile([128, 2 * NXB], proj_dt, tag="hqT")
                nc.vector.tensor_copy(
                    hqT_sb[:].rearrange("p (t n) -> p t n", t=2)[:, :, 0:NX_IO],
                    hqI_sb[:].rearrange("p (t n) -> p t n", t=2))
                if NF_IO < NFB:
                    for t in range(4):
                        nc.vector.memset(
                            fkT_sb[:, t * NFB + NF_IO:(t + 1) * NFB].bitcast(f32), 0.0)
                if NX_IO < NXB:
                    for t in range(2):
                        nc.vector.memset(
                            hqT_sb[:, t * NXB + NX_IO:(t + 1) * NXB].bitcast(f32), 0.0)

                v_sb = workp.tile([128, NJT * VW], bf16, tag="v")
                eT_sb = workp.tile([128, NJT * NXB], bf16, tag="eT")

                def v_proj(jt):
                    v_ps = psV.tile([128, VW], f32, tag="vps")
                    for t in range(4):
                        nc.tensor.matmul(
                            v_ps[:, 0:256],
                            lhsT=fkT_sb[:, t * NFB + jt * 128:t * NFB + jt * 128 + 128],
                            rhs=wv_sb[:, t * 256:(t + 1) * 256],
                            start=(t == 0), stop=(t == 3))
                    nc.vector.tensor_copy(v_sb[:, jt * VW:jt * VW + 256],
                                          v_ps[:, 0:256])
                    nc.vector.memset(v_sb[:, jt * VW + 256:jt * VW + VW], 1.0)

                # key projection, transposed: kT[d, j]
                kT_ps = psS.tile([128, PSW], f32, tag="big")
                for t in range(4):
                    for n0, nn, ps in nchFps:
                        nc.tensor.matmul(
                            kT_ps[:, ps:ps + nn],
                            lhsT=wk_sb[:, t * 128:(t + 1) * 128],
                            rhs=fkT_sb[:, t * NFB + n0:t * NFB + n0 + nn],
                            start=(t == 0), stop=(t == 3))
                kT_sb = workp.tile([128, NFB], score_dt, tag="kT")
                for n0, nn, ps in nchFps:
                    nc.scalar.copy(kT_sb[:, n0:n0 + nn], kT_ps[:, ps:ps + nn])
                v_proj(0)  # fills PE while the kT copy drains

                # query projection, transposed: qT[d, i]
                qT_ps = psS.tile([128, PSW], f32, tag="big")
                for t in range(2):
                    for n0, nn, ps in nchXps:
                        nc.tensor.matmul(
                            qT_ps[:, ps:ps + nn],
                            lhsT=wh_sb[:, t * 128:(t + 1) * 128],
                            rhs=hqT_sb[:, t * NXB + n0:t * NXB + n0 + nn],
                            start=(t == 0), stop=(t == 1))
                qT_sb = workp.tile([128, NXB], score_dt, tag="qT")
                for n0, nn, ps in nchXps:
                    nc.scalar.copy(qT_sb[:, n0:n0 + nn], qT_ps[:, ps:ps + nn])
                v_proj(1)

                # scores + exp per key tile, with the context accumulation
                # (previous tile) and value projection interleaved on PE
                NIC_IN = min(3, NIC)  # ctx columns accumulated inside the loop
                c_ps = [psCx.tile([128, VW], f32, tag="ctx", name=f"cps{b}_{i}")
                        for i in range(NIC_IN)]
                for jt in range(NJT):
                    sT_ps = psS.tile([128, PSW], f32, tag="big")
                    for n0, nn, ps in nchXps:
                        nc.tensor.matmul(
                            sT_ps[:, ps:ps + nn],
                            lhsT=kT_sb[:, jt * 128:(jt + 1) * 128],
                            rhs=qT_sb[:, n0:n0 + nn],
                            start=True, stop=True)
                    for n0, nn, ps in nchXps:
                        nc.scalar.activation(
                            eT_sb[:, jt * NXB + n0:jt * NXB + n0 + nn],
                            sT_ps[:, ps:ps + nn],
                            Exp, bias=cb_sb[:, b:b + 1], scale=1.0)
                    if jt + 2 < NJT:
                        v_proj(jt + 2)
                    for ic in range(NIC_IN):
                        nc.tensor.matmul(
                            c_ps[ic][:, 0:VW],
                            lhsT=eT_sb[:, jt * NXB + ic * 128:jt * NXB + ic * 128 + 128],
                            rhs=v_sb[:, jt * VW:(jt + 1) * VW],
                            start=(jt == 0), stop=(jt == NJT - 1))
                for ic in range(NIC_IN, NIC):
                    cp = psCx.tile([128, VW], f32, tag="ctx", name=f"cpsl{b}_{ic}")
                    c_ps.append(cp)
                    for jt in range(NJT):
                        nc.tensor.matmul(
                            cp[:, 0:VW],
                            lhsT=eT_sb[:, jt * NXB + ic * 128:jt * NXB + ic * 128 + 128],
                            rhs=v_sb[:, jt * VW:(jt + 1) * VW],
                            start=(jt == 0), stop=(jt == NJT - 1))

                # context epilogue: per-row reciprocal of the ones-column sum
                cx_all = outp.tile([128, NIC * 256], bf16, tag="cx")
                rc_all = workp.tile([128, NIC], f32, tag="rc")
                for ic in range(NIC):
                    nc.vector.reciprocal(rc_all[:, ic:ic + 1], c_ps[ic][:, 256:VW])
                    nc.vector.tensor_scalar(cx_all[:, ic * 256:(ic + 1) * 256],
                                            c_ps[ic][:, 0:256], rc_all[:, ic:ic + 1],
                                            None, mult)
                nc.scalar.dma_start(
                    out=ctx_d[b * NXB:(b + 1) * NXB, :].rearrange(
                        "(t p) n -> p t n", p=128),
                    in_=cx_all[:].rearrange("p (t n) -> p t n", t=NIC))

                # transpose per-query recips into a [1, NXB] row via PE,
                # broadcast across partitions, multiply, ship per key tile
                n1 = min(512, NXB)
                rcT_ps1 = psV.tile([1, n1], f32, tag="vps")
                rcT_ps2 = (psCx.tile([1, NXB - n1], f32, tag="ctx",
                                     name=f"rcT2_{b}")
                           if NXB > n1 else None)
                for ic in range(NIC):
                    if ic * 128 < n1:
                        dst = rcT_ps1[0:1, ic * 128:(ic + 1) * 128]
                    else:
                        dst = rcT_ps2[0:1, ic * 128 - n1:(ic + 1) * 128 - n1]
                    nc.tensor.matmul(dst, lhsT=rc_all[:, ic:ic + 1], rhs=id_sb[:],
                                     is_transpose=True, start=True, stop=True)
                rcT_sb = workp.tile([1, NXB], bf16, tag="rcT")
                nc.vector.tensor_copy(rcT_sb[0:1, 0:n1], rcT_ps1[:])
                if rcT_ps2 is not None:
                    nc.vector.tensor_copy(rcT_sb[0:1, n1:NXB], rcT_ps2[:])
                bc_sb = workp.tile([128, NXB], bf16, tag="bc")
                nc.gpsimd.partition_broadcast(bc_sb[:], rcT_sb[:])
                for jt in range(NJT):
                    at_sb = atp.tile([128, NXB], bf16, tag="at")
                    nc.vector.tensor_tensor(
                        at_sb[:, 0:NX_IO], eT_sb[:, jt * NXB:jt * NXB + NX_IO],
                        bc_sb[:, 0:NX_IO], mult)
                    nc.scalar.dma_start(
                        out=attnT_d[b * NFB + jt * 128:b * NFB + (jt + 1) * 128,
                                    0:NX_IO],
                        in_=at_sb[:, 0:NX_IO])

    nc.compile()
    return nc


def _get_compiled(NXB, NFB, NX_IO, NF_IO):
    key = (NXB, NFB, NX_IO, NF_IO, RMODE)
    if key not in _compiled_cache:
        _compiled_cache[key] = _build(NXB, NFB, NX_IO, NF_IO, RMODE)
    return _compiled_cache[key]


def _host_reference(f, h, xb, fb, W_h, W_key, W_value):
    """Exact numpy mirror of the reference; safety net for inputs far
    outside the compiled kernel's regime."""
    q = h @ W_h
    k = f @ W_key
    v = f @ W_value
    s = q @ k.T
    mask = xb[:, None] != fb[None, :]
    s = np.where(mask, np.float32(-1e9), s).astype(np.float32)
    m = s.max(axis=1, keepdims=True)
    e = np.exp(s - m)
    attn = e / e.sum(axis=1, keepdims=True)
    ctx = attn @ v
    return ctx.astype(np.float32), attn.astype(np.float32)


def kernel(f, x, h, edge_index, edge_type, f_batch, x_batch, x_init,
           W_h, W_key, W_value):
    from concourse.bass_utils import run_bass_kernel_spmd

    f = np.ascontiguousarray(np.asarray(f, dtype=np.float32))
    h = np.ascontiguousarray(np.asarray(h, dtype=np.float32))
    W_h = np.ascontiguousarray(np.asarray(W_h, dtype=np.float32))
    W_key = np.ascontiguousarray(np.asarray(W_key, dtype=np.float32))
    W_value = np.ascontiguousarray(np.asarray(W_value, dtype=np.float32))
    xb = np.asarray(x_batch).astype(np.int64)
    fb = np.asarray(f_batch).astype(np.int64)

    gids = np.arange(N_GRAPHS)
    xs = np.searchsorted(xb, gids, side="left")
    xe = np.searchsorted(xb, gids, side="right")
    fs = np.searchsorted(fb, gids, side="left")
    fe = np.searchsorted(fb, gids, side="right")
    nx_g = xe - xs
    nf_g = fe - fs

    NXB = _round_up(max(int(nx_g.max()), 1), 128)
    NFB = _round_up(max(int(nf_g.max()), 1), 128)
    NX_IO = _round_up(max(int(nx_g.max()), 1), 16)
    NF_IO = _round_up(max(int(nf_g.max()), 1), 16)

    # SBUF feasibility estimate (bytes/partition); fall back to a host
    # computation for pathological graph distributions far outside the
    # problem's regime (never triggers for the reference distribution)
    njt = NFB // 128
    sbuf_pp = (2 * (4 * NF_IO * 2 + 2 * NX_IO * 2 + 4 * NFB * 4 + 2 * NXB * 4)
               + 2 * (NFB * 4 + NXB * 4 + njt * 257 * 2 + njt * NXB * 2
                      + NXB * 2 + NXB * 2)
               + 5 * NXB * 2 + 2 * (NXB // 128) * 256 * 2 + 8192)
    sorted_ok = bool(np.all(np.diff(xb) >= 0) and np.all(np.diff(fb) >= 0))
    if sbuf_pp > 170 * 1024 or not sorted_ok:
        return _host_reference(f, h, xb, fb, W_h, W_key, W_value)

    # exact per-block score max for the exp shift (cheap on host via BLAS)
    q_host = h @ W_h
    k_host = f @ W_key
    cmax = np.zeros(N_GRAPHS, dtype=np.float32)
    for g in range(N_GRAPHS):
        if nx_g[g] > 0 and nf_g[g] > 0:
            cmax[g] = (q_host[xs[g]:xe[g]] @ k_host[fs[g]:fe[g]].T).max()

    nc = _get_compiled(NXB, NFB, NX_IO, NF_IO)

    scale_f = np.float32(max(np.abs(f).max(), 1e-30) / 32767.0)
    scale_h = np.float32(max(np.abs(h).max(), 1e-30) / 32767.0)
    f_q = np.round(f / scale_f).astype(np.int16)
    h_q = np.round(h / scale_h).astype(np.int16)
    wk_q = W_key * scale_f
    wh_q = W_h * scale_h
    wv_q = W_value * scale_f
    # the exp shift must track the quantized scores the device computes
    q_q = (h_q.astype(np.float32) * scale_h) @ W_h
    k_q = (f_q.astype(np.float32) * scale_f) @ W_key
    for g in range(N_GRAPHS):
        if nx_g[g] > 0 and nf_g[g] > 0:
            cmax[g] = (q_q[xs[g]:xe[g]] @ k_q[fs[g]:fe[g]].T).max()

    in_maps = []
    ident = np.eye(128, dtype=np.float32)
    for c in range(NCORES):
        hqT = np.zeros((OUT_SIZE, NB * NXB), np.int16)
        fkT = np.zeros((F_SIZE, NB * NFB), np.int16)
        cb = np.zeros((128, NB), np.float32)
        for b in range(NB):
            g = NB * c + b
            hqT[:, b * NXB:b * NXB + nx_g[g]] = h_q[xs[g]:xe[g]].T
            fkT[:, b * NFB:b * NFB + nf_g[g]] = f_q[fs[g]:fe[g]].T
            cb[:, b] = -cmax[g]
        in_maps.append({
            "hqT": hqT,
            "fkT": fkT,
            "w_h": wh_q,
            "w_key": wk_q,
            "w_vh": wv_q,
            "cbias": cb,
            "ident": ident,
        })

    res = run_bass_kernel_spmd(nc, in_maps, list(range(NCORES)))
    kernel._last_results = res

    attn = np.zeros((NX, NF), np.float32)
    ctx = np.zeros((NX, OUT_SIZE), np.float32)
    value_host = None
    for c in range(NCORES):
        at = np.asarray(res.results[c]["attnT"])
        cx = np.asarray(res.results[c]["ctx"])
        for b in range(NB):
            g = NB * c + b
            if nx_g[g] == 0:
                continue
            if nf_g[g] > 0:
                attn[xs[g]:xe[g], fs[g]:fe[g]] = \
                    at[b * NFB:b * NFB + nf_g[g], :nx_g[g]].T
                ctx[xs[g]:xe[g], :] = cx[b * NXB:b * NXB + nx_g[g], :]
            else:
                # graph with queries but no keys: reference softmax over an
                # all -1e9 row is uniform over ALL keys
                attn[xs[g]:xe[g], :] = np.float32(1.0) / np.float32(NF)
                if value_host is None:
                    value_host = f @ W_value
                ctx[xs[g]:xe[g], :] = value_host.mean(axis=0, dtype=np.float32)
    return ctx, attn


# revision 31
# speedup vs baseline: 1.2063x; 1.0447x over previous
"""Bass/Trainium2 kernel for nn_AttBlockMinimal (block-diagonal masked attention).

reference semantics:
    q = h @ W_h; k = f @ W_key; v = f @ W_value
    scores = q @ k.T  masked to -1e9 where x_batch[i] != f_batch[j] (block diagonal,
    both batch vectors sorted), softmax over keys, context = attn @ v.
    Returns (context [8192,256], attn [8192,8192]).

Masked softmax entries underflow to exactly 0.0, so attn is block-diagonal with
16 dense blocks (one per graph). Strategy: graph-parallel over 8 cores, 2 graphs
per core. The host slices/pads/transposes each graph's rows, every core runs the
same SPMD program on its two dense blocks, and the host scatters the dense
results into the (mostly zero) full outputs.

Device program per block (all matmuls contract over the partition dim):
    kT = W_key.T @ fT        [128d, NFB]
    qT = W_h.T  @ hT         [128d, NXB]
    v  = fT.T   @ W_value    [NFB, 257]    bf16, col 256 = 1 (gives ctx row sums)
    sT = kT.T-slices @ qT    [NFB, NXB]    (scores, transposed)
    e' = exp(sT - C_b)       bf16          (C_b = exact block max score, host side)
    rowsum = ones.T @ e'     [1, NXB], partition-broadcast to [128, NXB] (GpSimd)
    attnT  = e' / bcast(rowsum)  -> DRAM (host transposes per block)
    ctx_ps = e'.T @ v_aug;  ctx = ctx_ps[:, :256] * recip(ctx_ps[:, 256])

rmode: "fp32" - score path in fp32 (4 cyc/row), most accurate
       "sT"   - score matmul in f32r (~tf32), projections fp32
       "all"  - projections + score matmul in f32r
"""

import os
import numpy as np
import ml_dtypes

NX, NF = 8192, 8192
F_SIZE, OUT_SIZE, D_ATT = 512, 256, 128
N_GRAPHS = 16
NCORES = 8
NB = 2  # graph blocks per core
RMODE = os.environ.get("KERNEL_RMODE", "i16r")
N_WARM = 28  # HAM warm-up matmuls

_compiled_cache = {}


def _round_up(v, m):
    return ((v + m - 1) // m) * m


def _nchunks(n, c=512):
    out = []
    n0 = 0
    while n0 < n:
        nn = min(c, n - n0)
        out.append((n0, nn))
        n0 += nn
    return out


def _nchunks_ps(n):
    """(src_off, width, psum_off) chunks at 512-aligned psum offsets (one
    bank per matmul). Chunks are EQUAL so the chunked psum region is a
    regular strided AP (single activation/copy reads all chunks at once);
    width is kept >=256 where possible for full-rate f32r."""
    if n <= 512:
        return [(0, n, 0)]
    c = (n + 511) // 512
    w = n // c
    assert w * c == n and w % 32 == 0, f"unsupported width {n}"
    return [(i * w, w, i * 512) for i in range(c)]


def _build(NXB, NFB, NX_IO, NF_IO, rmode):
    import concourse.bacc as bacc
    import concourse.mybir as mybir
    from concourse import tile

    f32 = mybir.dt.float32
    f32r = mybir.dt.float32r
    bf16 = mybir.dt.bfloat16
    i16 = mybir.dt.int16
    Exp = mybir.ActivationFunctionType.Exp
    mult = mybir.AluOpType.mult
    divide = mybir.AluOpType.divide

    f16 = mybir.dt.float16
    if rmode == "f16":
        in_dt = proj_dt = score_dt = val_dt = f16
    elif rmode == "i16r":
        in_dt = i16
        proj_dt = score_dt = val_dt = f32r
    else:
        in_dt = proj_dt = f32r if rmode == "all" else f32
        score_dt = f32r if rmode in ("sT", "all") else f32
        val_dt = bf16

    NJT = NFB // 128  # key tiles per block
    NIC = NXB // 128  # query chunks per block
    VW = OUT_SIZE + 1  # value width + ones column
    nchX = _nchunks(NXB)
    nchXps = _nchunks_ps(NXB)
    nchFps = _nchunks_ps(NFB)
    PSW = max(nchXps[-1][2] + 512, nchFps[-1][2] + 512)

    nc = bacc.Bacc("TRN2", target_bir_lowering=False, debug=False,
                   num_devices=NCORES)
    hqT_d = nc.dram_tensor("hqT", [OUT_SIZE, NB * NXB], in_dt, kind="ExternalInput")
    fkT_d = nc.dram_tensor("fkT", [F_SIZE, NB * NFB], in_dt, kind="ExternalInput")
    wh_d = nc.dram_tensor("w_h", [OUT_SIZE, D_ATT], proj_dt, kind="ExternalInput")
    wk_d = nc.dram_tensor("w_key", [F_SIZE, D_ATT], proj_dt, kind="ExternalInput")
    wv_d = nc.dram_tensor("w_vh", [F_SIZE, OUT_SIZE], val_dt, kind="ExternalInput")
    cb_d = nc.dram_tensor("cbias", [128, NB], f32, kind="ExternalInput")
    id_d = nc.dram_tensor("ident", [128, 128], f32, kind="ExternalInput")
    attnT_d = nc.dram_tensor("attnT", [NB * NFB, NXB], bf16, kind="ExternalOutput")
    ctx_d = nc.dram_tensor("ctx", [NB * NXB, OUT_SIZE], bf16, kind="ExternalOutput")

    def dma_3d(dst_tile, src, nt, width, b):
        # one DMA: dram [(nt*128), NB*width] block-column b -> sbuf [128, nt*width]
        nc.sync.dma_start(
            out=dst_tile[:].rearrange("p (t n) -> p t n", t=nt),
            in_=src.rearrange("(t p) n -> p t n", p=128)[:, :, b * width:(b + 1) * width])

    with tile.TileContext(nc) as tc:
        with (
            tc.tile_pool(name="const", bufs=1) as constp,
            tc.tile_pool(name="io", bufs=2) as iop,
            tc.tile_pool(name="work", bufs=2) as workp,
            tc.tile_pool(name="out", bufs=2) as outp,
            tc.tile_pool(name="atp", bufs=5) as atp,
            tc.tile_pool(name="psS", bufs=2, space="PSUM") as psS,
            tc.tile_pool(name="psV", bufs=1, space="PSUM") as psV,
            tc.tile_pool(name="psCx", bufs=3, space="PSUM") as psCx,
        ):
            # HAM warm-up burst from a memset tile (no DMA dependency)
            warm_sb = constp.tile([128, 128], bf16, tag="warm")
            nc.vector.memset(warm_sb[:], 1.0)
            w_ps = psS.tile([128, PSW], f32, tag="big")
            for w in range(N_WARM):
                nc.tensor.matmul(w_ps[:, 0:128], lhsT=warm_sb[:], rhs=warm_sb[:],
                                 start=True, stop=True)

            wk_sb = constp.tile([128, 4 * 128], proj_dt, tag="wk")
            wh_sb = constp.tile([128, 2 * 128], proj_dt, tag="wh")
            wv_sb = constp.tile([128, 4 * 256], val_dt, tag="wv")
            cb_sb = constp.tile([128, NB], f32, tag="cb")
            id_sb = constp.tile([128, 128], f32, tag="ident")
            ones_b = constp.tile([1, 128], bf16, tag="onesb")
            nc.vector.memset(ones_b[:], 1.0)

            for b in range(NB):
                # int16 inputs: half the DMA bytes of f32r at better-than-bf16
                # precision; dequant scale is folded into the weights host-side
                fkI_sb = iop.tile([128, 4 * NF_IO], in_dt, tag="fkI")
                nc.sync.dma_start(
                    out=fkI_sb[:].rearrange("p (t n) -> p t n", t=4),
                    in_=fkT_d.rearrange("(t p) n -> p t n", p=128)
                        [:, :, b * NFB:b * NFB + NF_IO])
                if b == 0:
                    # small weights ride behind the critical first key input
                    nc.sync.dma_start(
                        out=wk_sb[:].rearrange("p (t n) -> p t n", t=4),
                        in_=wk_d.rearrange("(t p) n -> p t n", p=128))
                hqI_sb = iop.tile([128, 2 * NX_IO], in_dt, tag="hqI")
                nc.sync.dma_start(
                    out=hqI_sb[:].rearrange("p (t n) -> p t n", t=2),
                    in_=hqT_d.rearrange("(t p) n -> p t n", p=128)
                        [:, :, b * NXB:b * NXB + NX_IO])
                if b == 0:
                    nc.sync.dma_start(
                        out=wh_sb[:].rearrange("p (t n) -> p t n", t=2),
                        in_=wh_d.rearrange("(t p) n -> p t n", p=128))
                    nc.sync.dma_start(out=cb_sb[:, :], in_=cb_d[:, :])
                    nc.sync.dma_start(
                        out=wv_sb[:].rearrange("p (t n) -> p t n", t=4),
                        in_=wv_d.rearrange("(t p) n -> p t n", p=128))
                    nc.sync.dma_start(out=id_sb[:], in_=id_d[:, :])
                # dequantize to f32r (the int16->float convert rounds; the
                # quant scale is folded into the weights host-side)
                fkT_sb = iop.tile([128, 4 * NFB], proj_dt, tag="fkT")
                for t in range(4):
                    nc.vector.tensor_copy(
                        fkT_sb[:, t * NFB:t * NFB + NF_IO],
                        fkI_sb[:, t * NF_IO:(t + 1) * NF_IO])
                hqT_sb = iop.tile([128, 2 * NXB], proj_dt, tag="hqT")
                nc.vector.tensor_copy(
                    hqT_sb[:].rearrange("p (t n) -> p t n", t=2)[:, :, 0:NX_IO],
                    hqI_sb[:].rearrange("p (t n) -> p t n", t=2))
                if NF_IO < NFB:
                    for t in range(4):
                        nc.vector.memset(
                            fkT_sb[:, t * NFB + NF_IO:(t + 1) * NFB].bitcast(f32), 0.0)
                if NX_IO < NXB:
                    for t in range(2):
                        nc.vector.memset(
                            hqT_sb[:, t * NXB + NX_IO:(t + 1) * NXB].bitcast(f32), 0.0)

                v_sb = workp.tile([128, NJT * VW], bf16, tag="v")
                eT_sb = workp.tile([128, NJT * NXB], bf16, tag="eT")

                def v_proj(jt):
                    v_ps = psV.tile([128, VW], f32, tag="vps")
                    for t in range(4):
                        nc.tensor.matmul(
                            v_ps[:, 0:256],
                            lhsT=fkT_sb[:, t * NFB + jt * 128:t * NFB + jt * 128 + 128],
                            rhs=wv_sb[:, t * 256:(t + 1) * 256],
                            start=(t == 0), stop=(t == 3))
                    nc.vector.tensor_copy(v_sb[:, jt * VW:jt * VW + 256],
                                          v_ps[:, 0:256])
                    nc.vector.memset(v_sb[:, jt * VW + 256:jt * VW + VW], 1.0)

                # key projection, transposed: kT[d, j]
                kT_ps = psS.tile([128, PSW], f32, tag="big")
                for t in range(4):
                    for n0, nn, ps in nchFps:
                        nc.tensor.matmul(
                            kT_ps[:, ps:ps + nn],
                            lhsT=wk_sb[:, t * 128:(t + 1) * 128],
                            rhs=fkT_sb[:, t * NFB + n0:t * NFB + n0 + nn],
                            start=(t == 0), stop=(t == 3))
                kT_sb = workp.tile([128, NFB], score_dt, tag="kT")
                wF = nchFps[0][1]
                if len(nchFps) > 1:
                    nc.scalar.copy(
                        kT_sb[:].rearrange("p (c w) -> p c w", w=wF),
                        kT_ps[:, 0:len(nchFps) * 512].rearrange(
                            "p (c w) -> p c w", w=512)[:, :, 0:wF])
                else:
                    nc.scalar.copy(kT_sb[:], kT_ps[:, 0:NFB])
                v_proj(0)  # fills PE while the kT copy drains

                # query projection, transposed: qT[d, i]
                qT_ps = psS.tile([128, PSW], f32, tag="big")
                for t in range(2):
                    for n0, nn, ps in nchXps:
                        nc.tensor.matmul(
                            qT_ps[:, ps:ps + nn],
                            lhsT=wh_sb[:, t * 128:(t + 1) * 128],
                            rhs=hqT_sb[:, t * NXB + n0:t * NXB + n0 + nn],
                            start=(t == 0), stop=(t == 1))
                qT_sb = workp.tile([128, NXB], score_dt, tag="qT")
                wX = nchXps[0][1]
                if len(nchXps) > 1:
                    nc.scalar.copy(
                        qT_sb[:].rearrange("p (c w) -> p c w", w=wX),
                        qT_ps[:, 0:len(nchXps) * 512].rearrange(
                            "p (c w) -> p c w", w=512)[:, :, 0:wX])
                else:
                    nc.scalar.copy(qT_sb[:], qT_ps[:, 0:NXB])
                v_proj(1)

                # scores + exp per key tile, with the context accumulation
                # (previous tile) and value projection interleaved on PE
                NIC_IN = min(3, NIC)  # ctx columns accumulated inside the loop
                c_ps = [psCx.tile([128, VW], f32, tag="ctx", name=f"cps{b}_{i}")
                        for i in range(NIC_IN)]
                for jt in range(NJT):
                    sT_ps = psS.tile([128, PSW], f32, tag="big")
                    for n0, nn, ps in nchXps:
                        nc.tensor.matmul(
                            sT_ps[:, ps:ps + nn],
                            lhsT=kT_sb[:, jt * 128:(jt + 1) * 128],
                            rhs=qT_sb[:, n0:n0 + nn],
                            start=True, stop=True)
                    if len(nchXps) > 1:
                        nc.scalar.activation(
                            eT_sb[:, jt * NXB:(jt + 1) * NXB].rearrange(
                                "p (c w) -> p c w", w=wX),
                            sT_ps[:, 0:len(nchXps) * 512].rearrange(
                                "p (c w) -> p c w", w=512)[:, :, 0:wX],
                            Exp, bias=cb_sb[:, b:b + 1], scale=1.0)
                    else:
                        nc.scalar.activation(
                            eT_sb[:, jt * NXB:(jt + 1) * NXB], sT_ps[:, 0:NXB],
                            Exp, bias=cb_sb[:, b:b + 1], scale=1.0)
                    if jt + 2 < NJT:
                        v_proj(jt + 2)
                    for ic in range(NIC_IN):
                        nc.tensor.matmul(
                            c_ps[ic][:, 0:VW],
                            lhsT=eT_sb[:, jt * NXB + ic * 128:jt * NXB + ic * 128 + 128],
                            rhs=v_sb[:, jt * VW:(jt + 1) * VW],
                            start=(jt == 0), stop=(jt == NJT - 1))
                for ic in range(NIC_IN, NIC):
                    cp = psCx.tile([128, VW], f32, tag="ctx", name=f"cpsl{b}_{ic}")
                    c_ps.append(cp)
                    for jt in range(NJT):
                        nc.tensor.matmul(
                            cp[:, 0:VW],
                            lhsT=eT_sb[:, jt * NXB + ic * 128:jt * NXB + ic * 128 + 128],
                            rhs=v_sb[:, jt * VW:(jt + 1) * VW],
                            start=(jt == 0), stop=(jt == NJT - 1))

                # context epilogue: per-row reciprocal of the ones-column sum
                cx_all = outp.tile([128, NIC * 256], bf16, tag="cx")
                rc_all = workp.tile([128, NIC], f32, tag="rc")
                for ic in range(NIC):
                    nc.vector.reciprocal(rc_all[:, ic:ic + 1], c_ps[ic][:, 256:VW])
                    nc.vector.tensor_scalar(cx_all[:, ic * 256:(ic + 1) * 256],
                                            c_ps[ic][:, 0:256], rc_all[:, ic:ic + 1],
                                            None, mult)
                nc.scalar.dma_start(
                    out=ctx_d[b * NXB:(b + 1) * NXB, :].rearrange(
                        "(t p) n -> p t n", p=128),
                    in_=cx_all[:].rearrange("p (t n) -> p t n", t=NIC))

                # transpose per-query recips into a [1, NXB] row via PE,
                # broadcast across partitions, multiply, ship per key tile
                n1 = min(512, NXB)
                rcT_ps1 = psV.tile([1, n1], f32, tag="vps")
                rcT_ps2 = (psCx.tile([1, NXB - n1], f32, tag="ctx",
                                     name=f"rcT2_{b}")
                           if NXB > n1 else None)
                for ic in range(NIC):
                    if ic * 128 < n1:
                        dst = rcT_ps1[0:1, ic * 128:(ic + 1) * 128]
                    else:
                        dst = rcT_ps2[0:1, ic * 128 - n1:(ic + 1) * 128 - n1]
                    nc.tensor.matmul(dst, lhsT=rc_all[:, ic:ic + 1], rhs=id_sb[:],
                                     is_transpose=True, start=True, stop=True)
                rcT_sb = workp.tile([1, NXB], bf16, tag="rcT")
                nc.vector.tensor_copy(rcT_sb[0:1, 0:n1], rcT_ps1[:])
                if rcT_ps2 is not None:
                    nc.vector.tensor_copy(rcT_sb[0:1, n1:NXB], rcT_ps2[:])
                bc_sb = workp.tile([128, NXB], bf16, tag="bc")
                nc.gpsimd.partition_broadcast(bc_sb[:], rcT_sb[:])
                for jt in range(NJT):
                    at_sb = atp.tile([128, NXB], bf16, tag="at")
                    nc.vector.tensor_tensor(
                        at_sb[:, 0:NX_IO], eT_sb[:, jt * NXB:jt * NXB + NX_IO],
                        bc_sb[:, 0:NX_IO], mult)
                    nc.scalar.dma_start(
                        out=attnT_d[b * NFB + jt * 128:b * NFB + (jt + 1) * 128,
                                    0:NX_IO],
                        in_=at_sb[:, 0:NX_IO])

    nc.compile()
    return nc


def _get_compiled(NXB, NFB, NX_IO, NF_IO):
    key = (NXB, NFB, NX_IO, NF_IO, RMODE)
    if key not in _compiled_cache:
        _compiled_cache[key] = _build(NXB, NFB, NX_IO, NF_IO, RMODE)
    return _compiled_cache[key]


def _host_reference(f, h, xb, fb, W_h, W_key, W_value):
    """Exact numpy mirror of the reference; safety net for inputs far
    outside the compiled kernel's regime."""
    q = h @ W_h
    k = f @ W_key
    v = f @ W_value
    s = q @ k.T
    mask = xb[:, None] != fb[None, :]
    s = np.where(mask, np.float32(-1e9), s).astype(np.float32)
    m = s.max(axis=1, keepdims=True)
    e = np.exp(s - m)
    attn = e / e.sum(axis=1, keepdims=True)
    ctx = attn @ v
    return ctx.astype(np.float32), attn.astype(np.float32)


def kernel(f, x, h, edge_index, edge_type, f_batch, x_batch, x_init,
           W_h, W_key, W_value):
    from concourse.bass_utils import run_bass_kernel_spmd

    f = np.ascontiguousarray(np.asarray(f, dtype=np.float32))
    h = np.ascontiguousarray(np.asarray(h, dtype=np.float32))
    W_h = np.ascontiguousarray(np.asarray(W_h, dtype=np.float32))
    W_key = np.ascontiguousarray(np.asarray(W_key, dtype=np.float32))
    W_value = np.ascontiguousarray(np.asarray(W_value, dtype=np.float32))
    xb = np.asarray(x_batch).astype(np.int64)
    fb = np.asarray(f_batch).astype(np.int64)

    gids = np.arange(N_GRAPHS)
    xs = np.searchsorted(xb, gids, side="left")
    xe = np.searchsorted(xb, gids, side="right")
    fs = np.searchsorted(fb, gids, side="left")
    fe = np.searchsorted(fb, gids, side="right")
    nx_g = xe - xs
    nf_g = fe - fs

    NXB = _round_up(max(int(nx_g.max()), 1), 128)
    NFB = _round_up(max(int(nf_g.max()), 1), 128)
    NX_IO = _round_up(max(int(nx_g.max()), 1), 16)
    NF_IO = _round_up(max(int(nf_g.max()), 1), 16)

    # SBUF feasibility estimate (bytes/partition); fall back to a host
    # computation for pathological graph distributions far outside the
    # problem's regime (never triggers for the reference distribution)
    njt = NFB // 128
    sbuf_pp = (2 * (4 * NF_IO * 2 + 2 * NX_IO * 2 + 4 * NFB * 4 + 2 * NXB * 4)
               + 2 * (NFB * 4 + NXB * 4 + njt * 257 * 2 + njt * NXB * 2
                      + NXB * 2 + NXB * 2)
               + 5 * NXB * 2 + 2 * (NXB // 128) * 256 * 2 + 8192)
    sorted_ok = bool(np.all(np.diff(xb) >= 0) and np.all(np.diff(fb) >= 0))
    if sbuf_pp > 170 * 1024 or not sorted_ok:
        return _host_reference(f, h, xb, fb, W_h, W_key, W_value)

    # exact per-block score max for the exp shift (cheap on host via BLAS)
    q_host = h @ W_h
    k_host = f @ W_key
    cmax = np.zeros(N_GRAPHS, dtype=np.float32)
    for g in range(N_GRAPHS):
        if nx_g[g] > 0 and nf_g[g] > 0:
            cmax[g] = (q_host[xs[g]:xe[g]] @ k_host[fs[g]:fe[g]].T).max()

    nc = _get_compiled(NXB, NFB, NX_IO, NF_IO)

    scale_f = np.float32(max(np.abs(f).max(), 1e-30) / 32767.0)
    scale_h = np.float32(max(np.abs(h).max(), 1e-30) / 32767.0)
    f_q = np.round(f / scale_f).astype(np.int16)
    h_q = np.round(h / scale_h).astype(np.int16)
    wk_q = W_key * scale_f
    wh_q = W_h * scale_h
    wv_q = W_value * scale_f
    # the exp shift must track the quantized scores the device computes
    q_q = (h_q.astype(np.float32) * scale_h) @ W_h
    k_q = (f_q.astype(np.float32) * scale_f) @ W_key
    for g in range(N_GRAPHS):
        if nx_g[g] > 0 and nf_g[g] > 0:
            cmax[g] = (q_q[xs[g]:xe[g]] @ k_q[fs[g]:fe[g]].T).max()

    in_maps = []
    ident = np.eye(128, dtype=np.float32)
    for c in range(NCORES):
        hqT = np.zeros((OUT_SIZE, NB * NXB), np.int16)
        fkT = np.zeros((F_SIZE, NB * NFB), np.int16)
        cb = np.zeros((128, NB), np.float32)
        for b in range(NB):
            g = NB * c + b
            hqT[:, b * NXB:b * NXB + nx_g[g]] = h_q[xs[g]:xe[g]].T
            fkT[:, b * NFB:b * NFB + nf_g[g]] = f_q[fs[g]:fe[g]].T
            cb[:, b] = -cmax[g]
        in_maps.append({
            "hqT": hqT,
            "fkT": fkT,
            "w_h": wh_q,
            "w_key": wk_q,
            "w_vh": wv_q,
            "cbias": cb,
            "ident": ident,
        })

    res = run_bass_kernel_spmd(nc, in_maps, list(range(NCORES)))
    kernel._last_results = res

    attn = np.zeros((NX, NF), np.float32)
    ctx = np.zeros((NX, OUT_SIZE), np.float32)
    value_host = None
    for c in range(NCORES):
        at = np.asarray(res.results[c]["attnT"])
        cx = np.asarray(res.results[c]["ctx"])
        for b in range(NB):
            g = NB * c + b
            if nx_g[g] == 0:
                continue
            if nf_g[g] > 0:
                attn[xs[g]:xe[g], fs[g]:fe[g]] = \
                    at[b * NFB:b * NFB + nf_g[g], :nx_g[g]].T
                ctx[xs[g]:xe[g], :] = cx[b * NXB:b * NXB + nx_g[g], :]
            else:
                # graph with queries but no keys: reference softmax over an
                # all -1e9 row is uniform over ALL keys
                attn[xs[g]:xe[g], :] = np.float32(1.0) / np.float32(NF)
                if value_host is None:
                    value_host = f @ W_value
                ctx[xs[g]:xe[g], :] = value_host.mean(axis=0, dtype=np.float32)
    return ctx, attn
